# revision 26
# baseline (speedup 1.0000x reference)
"""Trainium2 Bass kernel for sliding-window unfold (im2col).

reference:  out = x[:, idx, :]  with idx[w, f] = w + f
  x:   [128, 4096, 4]  f32
  out: [128, 4065, 32, 4]  f32

out[b, w] (= 128 floats = 512 B) is the contiguous slice
x[b].flat[4w : 4w + 128]; the problem is a sliding-window byte
replication and HBM write bandwidth is the roofline.  Per core
(16 batches): 33.3 MB of output writes through 16 SDMA engines capped
at ~26.85 GB/s each, shared by loads and stores.

WPP=62 layout: each bulk tile holds TWO batches (64 partitions each,
62 windows per partition), which cuts the overlapped x-load redundancy
from 2.0x (248 f32 loaded per 124 unique) to 1.5x (372 per 248):
loads drop 2.03 MB -> 1.52 MB, all of which rides the same engine pipe
as the stores.  Stores are issued as two 31-window column pieces per
tile so every store keeps the proven shape: 128 descriptors x 15872 B,
descriptor count ~ 0 mod 16 (uniform engine spray regardless of ring
cursor).  Tile 0's first piece is split 8+23 windows so the first
store still triggers ~12 us.  Tail (windows 3968..4064) unchanged.

Device-state warning (measured 2026-08-10): exec is BIMODAL run to
run.  ~50% of runs are clean; ~25% have SDMA engine E79 degraded to
~21.5 GB/s (+10-17 us); ~25% have a ~10% uniform slowdown on ALL
engines.  The degradation is NOT caused by this kernel, persists
across runs, accumulates until the device goes
NRT_EXEC_UNIT_UNRECOVERABLE, and a device reset clears it (an earlier
session's 111 us "baseline" and its E79 doctrine were measured
entirely on a degraded device).

SWDGE descriptor->engine model (verified by HW probe runs):
  - Each dma_start's descriptors are dealt to the 16 SDMA engines in
    contiguous chunks of ceil(N/16), starting at a global ring cursor
    g that advances with every store: by N_data + one 4B
    completion-sem descriptor per participating engine for
    SBUF-sourced stores, by N_data alone for DRAM->DRAM stores.
    All stores here have N=128 (+16 sems) so spray is uniform.
  - N=120 at g==0 skips one engine (chunk 8, lanes 0..14); ragged
    counts (N % chunk != 0, e.g. 113) hit a broken ucode path (all
    descs on one engine) - keep N % 16 == 0.
  - The Tile scheduler reorders same-queue DMAs by dependency
    readiness; add_dep_helper(sync=False) edges are NOT honored, so
    ring-cursor-sensitive store sequences cannot be pinned cheaply.
    Skewing bytes away from E79 (120+8 pairs) works when aligned but
    costs +1.5-2 us clean and only pays in ~25% of runs -> rejected.
  - Descriptors <= ~768 B aggregate into multi-desc packets; D2D
    stores with strided src run at ~10-13 GB/s/engine.  A zero-dep
    D2D tail store starves the HW-queue loads (+5 us) -> rejected.
  - Tile inserts WAW semaphores between DMAs with overlapping DRAM
    ranges - keep all output writes strictly disjoint.
  - Every dma_start costs ~0.6 us trigger time; completion semaphores
    fire ~2 us after the last byte.
  - Also tried, neutral-to-negative on clean runs: SWDGE warmup
    store, head-slice X0a load, store reorders, load-ring rebalance.
    The drain start is gated by load traffic occupying the engines
    until ~15 us, not by the first store's descriptor chain.
"""

import numpy as np

from concourse import bacc, mybir, tile
from concourse.bass_utils import run_bass_kernel_spmd

N_CORES = 8
B_FULL = 128
B = B_FULL // N_CORES  # 16 batches per core
S = 4096
C = 4
F = 32
W = S - F + 1    # 4065
FL = F * C       # 128 floats per window
XB = S * C       # 16384 floats per batch of x
OB = W * FL      # 520320 floats per batch of out

WPP = 62                   # windows per partition (2 batches per tile)
HB = 64                    # partitions per batch within a tile
NT = B // 2                # 8 bulk tiles
YROW = WPP * FL            # 7936 floats per partition row
XROW = (WPP - 1) * C + FL  # 372 floats of x per partition
PC = 31                    # windows per store/expand piece
PCE = PC * FL              # 3968 floats per piece per partition
NBULK = HB * WPP           # 3968 bulk windows per batch (as before)

# tail geometry: windows 3968..4063 as 8 slices of 12 windows per batch
# (partition p = 8*b + s, strictly disjoint writes); window 4064 is a
# [16, 128] raw load+store (partition = batch, contiguous 512 B rows).
TSL = 8                    # slices per batch
TWIN = 12                  # windows per slice
TSTR = 12                  # window stride between slices
TROW = TWIN * FL           # 1536 floats of tail output per partition
RLD = 176                  # floats of raw x loaded per partition
W4 = W - 1                 # window 4064
H0 = 8                     # windows in tile-0 first piece (small so the
                           # first store triggers ~12 us)
H1 = PC - H0               # 23 windows in the second piece (ACT)

_cache = {}


def build_nc():
    nc = bacc.Bacc("TRN2", target_bir_lowering=False)
    x = nc.dram_tensor("x", [B, S, C], mybir.dt.float32, kind="ExternalInput")
    out = nc.dram_tensor("out", [B, W, F, C], mybir.dt.float32, kind="ExternalOutput")

    with tile.TileContext(nc) as tc:
        with (
            tc.tile_pool(name="xp", bufs=NT) as xp,
            tc.tile_pool(name="yp", bufs=5) as yp,
            tc.tile_pool(name="rp", bufs=1) as rp,
            tc.tile_pool(name="vp", bufs=1) as vp,
            tc.tile_pool(name="tp", bufs=1) as tp,
        ):
            def ld(engine, dst_tile, dst_ap, dst_off, src_ap, src_off):
                src = x[:].copy()
                src.ap = mybir.VecI64Pair(src_ap)
                src.offset = src_off
                dst = dst_tile[:].copy()
                dst.ap = mybir.VecI64Pair(dst_ap)
                dst.offset = dst_off
                engine.dma_start(out=dst, in_=src)

            def st(engine, src_tile, src_ap, src_off, dst_ap, dst_off):
                dst = out[:].copy()
                dst.ap = mybir.VecI64Pair(dst_ap)
                dst.offset = dst_off
                src = src_tile[:].copy()
                src.ap = mybir.VecI64Pair(src_ap)
                src.offset = src_off
                return engine.dma_start(out=dst, in_=src)

            def expandp(engine, xt, yt, j0, nwin):
                # windows [j0, j0+nwin) of every partition's 62-window row
                src = xt[:].copy()
                src.ap = mybir.VecI64Pair([[XROW, 128], [C, nwin], [1, FL]])
                src.offset = j0 * C
                dst = yt[:].copy()
                dst.ap = mybir.VecI64Pair([[YROW, 128], [FL, nwin], [1, FL]])
                dst.offset = j0 * FL
                if engine is nc.vector:
                    engine.tensor_copy(out=dst, in_=src)
                else:
                    engine.copy(out=dst, in_=src)

            def stp(yt, t, i, j0, nwin):
                # store windows [j0, j0+nwin) of batch-half i of tile t:
                # 64 descriptors of nwin*FL f32.  2-dim dst with uniform
                # stride only - a 3-dim dst whose shape doesn't match the
                # src AP makes the ucode descriptor generator emit 4B
                # descriptors for the misaligned positions (measured:
                # 15k 4B packets, 2x exec).  64 + 16 sem descs = 80 = 0
                # mod 16, so the engine spray stays uniform in any order.
                return st(nc.gpsimd, yt,
                          [[YROW, HB], [1, nwin * FL]],
                          i * HB * YROW + j0 * FL,
                          [[YROW, HB], [1, nwin * FL]],
                          (2 * t + i) * OB + j0 * FL)

            def ld_tile(engine, xt, t):
                # compact 2-batch load: partition 64*i+q gets
                # x[2t+i].flat[248*q : 248*q + 372]
                ld(engine, xt, [[XROW, 128], [1, XROW]], 0,
                   [[XB, 2], [WPP * C, HB], [1, XROW]], 2 * t * XB)

            # ---- loads ----
            # scalar ring: window-4064 raw load, tail raw load, tiles 4-7
            V = vp.tile([16, FL], mybir.dt.float32)
            ld(nc.scalar, V, [[FL, 16], [1, FL]], 0,
               [[XB, B], [1, FL]], W4 * C)
            R = rp.tile([128, RLD], mybir.dt.float32)
            ld(nc.scalar, R, [[RLD, 128], [1, RLD]], 0,
               [[XB, B], [TSTR * C, TSL], [1, RLD]], NBULK * C)
            Xs = []
            for t in range(NT):
                Xt = xp.tile([128, XROW], mybir.dt.float32, name="Xt")
                Xs.append(Xt)
            for t in range(4):
                ld_tile(nc.sync, Xs[t], t)
            for t in range(4, NT):
                ld_tile(nc.scalar, Xs[t], t)

            # ---- expands ----
            # DVE: tile-0 head piece, then every tile's second piece.
            # ACT: tile-0 second head piece, tail expand, then tiles
            # 1..7 first pieces.  ~35 us per engine, well under drain.
            Ys = [yp.tile([128, YROW], mybir.dt.float32, name="Yt")
                  for t in range(NT)]
            expandp(nc.vector, Xs[0], Ys[0], 0, H0)
            expandp(nc.scalar, Xs[0], Ys[0], H0, H1)
            T = tp.tile([128, TROW], mybir.dt.float32)
            tsrc = R[:].copy()
            tsrc.ap = mybir.VecI64Pair([[RLD, 128], [C, TWIN], [1, FL]])
            tsrc.offset = 0
            tdst = T[:].copy()
            tdst.ap = mybir.VecI64Pair([[TROW, 128], [FL, TWIN], [1, FL]])
            tdst.offset = 0
            nc.scalar.copy(out=tdst, in_=tsrc)
            for t in range(NT):
                expandp(nc.vector, Xs[t], Ys[t], PC, PC)
                if t >= 1:
                    expandp(nc.scalar, Xs[t], Ys[t], 0, PC)

            # ---- stores: ALL on GPSIMD/SWDGE, FIFO order by earliest
            # dependency.  Every store: 128 descriptors, disjoint dst.
            st(nc.gpsimd, V, [[FL, 16], [1, FL]], 0,
               [[OB, B], [1, FL]], W4 * FL)
            stp(Ys[0], 0, 0, 0, H0)
            stp(Ys[0], 0, 1, 0, H0)
            stp(Ys[0], 0, 0, H0, H1)
            stp(Ys[0], 0, 1, H0, H1)
            st(nc.gpsimd, T, [[TROW, 128], [1, TROW]], 0,
               [[OB, B], [TSTR * FL, TSL], [1, TROW]], NBULK * FL)
            stp(Ys[0], 0, 0, PC, PC)
            stp(Ys[0], 0, 1, PC, PC)
            # tiles 1-7: one full-row store per batch half.  Both sides
            # merge flat and the lowering re-splits to 32 descriptors of
            # 63488 B, so each engine gets two ADJACENT descriptors =
            # 127 KB of contiguous DRAM writes.  (Piece stores whose
            # descriptor stride exceeds the descriptor size scatter each
            # engine's writes and cost ~35% per-packet rate - measured.)
            for t in range(1, NT):
                for i in range(2):
                    st(nc.gpsimd, Ys[t],
                       [[YROW, HB], [1, YROW]], i * HB * YROW,
                       [[YROW, HB], [1, YROW]], (2 * t + i) * OB)

    nc.finalize()
    return nc


def run_sharded(x: np.ndarray, trace: bool = False):
    """Shard batch across 8 cores, run, gather. Returns (out, raw results)."""
    if "nc" not in _cache:
        _cache["nc"] = build_nc()
    nc = _cache["nc"]

    x = np.ascontiguousarray(x, dtype=np.float32)
    in_maps = [{"x": x[i * B : (i + 1) * B]} for i in range(N_CORES)]
    res = run_bass_kernel_spmd(nc, in_maps, list(range(N_CORES)), trace=trace)
    out = np.concatenate([res.results[i]["out"] for i in range(N_CORES)], axis=0)
    return out, res


def kernel(x: np.ndarray) -> np.ndarray:
    out, _ = run_sharded(x, trace=False)
    return out


# revision 27
# speedup vs baseline: 1.2438x; 1.2438x over previous
"""Trainium2 Bass kernel for sliding-window unfold (im2col).

reference:  out = x[:, idx, :]  with idx[w, f] = w + f
  x:   [128, 4096, 4]  f32
  out: [128, 4065, 32, 4]  f32

out[b, w] (= 128 floats = 512 B) is the contiguous slice
x[b].flat[4w : 4w + 128]; the problem is a sliding-window byte
replication and HBM write bandwidth is the roofline.  Per core
(16 batches): 33.3 MB of output writes through 16 SDMA engines capped
at ~26.85 GB/s each, shared by loads and stores.

WPP=62 layout: each bulk tile holds TWO batches (64 partitions each,
62 windows per partition), which cuts the overlapped x-load redundancy
from 2.0x (248 f32 loaded per 124 unique) to 1.5x (372 per 248):
loads drop 2.03 MB -> 1.52 MB, all of which rides the same engine pipe
as the stores.  Stores are issued as two 31-window column pieces per
tile so every store keeps the proven shape: 128 descriptors x 15872 B,
descriptor count ~ 0 mod 16 (uniform engine spray regardless of ring
cursor).  Tile 0's first piece is split 8+23 windows so the first
store still triggers ~12 us.  Tail (windows 3968..4064) unchanged.

Device-state warning (measured 2026-08-10): exec is BIMODAL run to
run.  ~50% of runs are clean; ~25% have SDMA engine E79 degraded to
~21.5 GB/s (+10-17 us); ~25% have a ~10% uniform slowdown on ALL
engines.  The degradation is NOT caused by this kernel, persists
across runs, accumulates until the device goes
NRT_EXEC_UNIT_UNRECOVERABLE, and a device reset clears it (an earlier
session's 111 us "baseline" and its E79 doctrine were measured
entirely on a degraded device).

SWDGE descriptor->engine model (verified by HW probe runs):
  - Each dma_start's descriptors are dealt to the 16 SDMA engines in
    contiguous chunks of ceil(N/16), starting at a global ring cursor
    g that advances with every store: by N_data + one 4B
    completion-sem descriptor per participating engine for
    SBUF-sourced stores, by N_data alone for DRAM->DRAM stores.
    All stores here have N=128 (+16 sems) so spray is uniform.
  - N=120 at g==0 skips one engine (chunk 8, lanes 0..14); ragged
    counts (N % chunk != 0, e.g. 113) hit a broken ucode path (all
    descs on one engine) - keep N % 16 == 0.
  - The Tile scheduler reorders same-queue DMAs by dependency
    readiness; add_dep_helper(sync=False) edges are NOT honored, so
    ring-cursor-sensitive store sequences cannot be pinned cheaply.
    Skewing bytes away from E79 (120+8 pairs) works when aligned but
    costs +1.5-2 us clean and only pays in ~25% of runs -> rejected.
  - Descriptors <= ~768 B aggregate into multi-desc packets; D2D
    stores with strided src run at ~10-13 GB/s/engine.  A zero-dep
    D2D tail store starves the HW-queue loads (+5 us) -> rejected.
  - Tile inserts WAW semaphores between DMAs with overlapping DRAM
    ranges - keep all output writes strictly disjoint.
  - Every dma_start costs ~0.6 us trigger time; completion semaphores
    fire ~2 us after the last byte.
  - Also tried, neutral-to-negative on clean runs: SWDGE warmup
    store, head-slice X0a load, store reorders, load-ring rebalance.
    The drain start is gated by load traffic occupying the engines
    until ~15 us, not by the first store's descriptor chain.
"""

import numpy as np

from concourse import bacc, mybir, tile
from concourse.bass_utils import run_bass_kernel_spmd

N_CORES = 8
B_FULL = 128
B = B_FULL // N_CORES  # 16 batches per core
S = 4096
C = 4
F = 32
W = S - F + 1    # 4065
FL = F * C       # 128 floats per window
XB = S * C       # 16384 floats per batch of x
OB = W * FL      # 520320 floats per batch of out

WPP = 62                   # windows per partition (2 batches per tile)
HB = 64                    # partitions per batch within a tile
NT = B // 2                # 8 bulk tiles
YROW = WPP * FL            # 7936 floats per partition row
XROW = (WPP - 1) * C + FL  # 372 floats of x per partition
PC = 31                    # windows per store/expand piece
PCE = PC * FL              # 3968 floats per piece per partition
NBULK = HB * WPP           # 3968 bulk windows per batch (as before)

# tail geometry: windows 3968..4063 as 8 slices of 12 windows per batch
# (partition p = 8*b + s, strictly disjoint writes); window 4064 is a
# [16, 128] raw load+store (partition = batch, contiguous 512 B rows).
TSL = 8                    # slices per batch
TWIN = 12                  # windows per slice
TSTR = 12                  # window stride between slices
TROW = TWIN * FL           # 1536 floats of tail output per partition
RLD = 176                  # floats of raw x loaded per partition
W4 = W - 1                 # window 4064
H0 = 8                     # windows in tile-0 first piece (small so the
                           # first store triggers ~12 us)
H1 = PC - H0               # 23 windows in the second piece (ACT)

_cache = {}


def build_nc():
    nc = bacc.Bacc("TRN2", target_bir_lowering=False)
    x = nc.dram_tensor("x", [B, S, C], mybir.dt.float32, kind="ExternalInput")
    out = nc.dram_tensor("out", [B, W, F, C], mybir.dt.float32, kind="ExternalOutput")

    with tile.TileContext(nc) as tc:
        with (
            tc.tile_pool(name="xp", bufs=1) as xp,
            tc.tile_pool(name="yp", bufs=5) as yp,
            tc.tile_pool(name="rp", bufs=1) as rp,
            tc.tile_pool(name="vp", bufs=1) as vp,
            tc.tile_pool(name="tp", bufs=1) as tp,
        ):
            def ld(engine, dst_tile, dst_ap, dst_off, src_ap, src_off):
                src = x[:].copy()
                src.ap = mybir.VecI64Pair(src_ap)
                src.offset = src_off
                dst = dst_tile[:].copy()
                dst.ap = mybir.VecI64Pair(dst_ap)
                dst.offset = dst_off
                engine.dma_start(out=dst, in_=src)

            def st(engine, src_tile, src_ap, src_off, dst_ap, dst_off):
                dst = out[:].copy()
                dst.ap = mybir.VecI64Pair(dst_ap)
                dst.offset = dst_off
                src = src_tile[:].copy()
                src.ap = mybir.VecI64Pair(src_ap)
                src.offset = src_off
                return engine.dma_start(out=dst, in_=src)

            def expandp(engine, xa, t, yt, j0, nwin):
                # windows [j0, j0+nwin) of every partition's 62-window row
                src = xa[:].copy()
                src.ap = mybir.VecI64Pair(
                    [[NT * XROW, 128], [C, nwin], [1, FL]])
                src.offset = t * XROW + j0 * C
                dst = yt[:].copy()
                dst.ap = mybir.VecI64Pair([[YROW, 128], [FL, nwin], [1, FL]])
                dst.offset = j0 * FL
                if engine is nc.vector:
                    engine.tensor_copy(out=dst, in_=src)
                else:
                    engine.copy(out=dst, in_=src)

            def stp(yt, t, i, j0, nwin):
                # store windows [j0, j0+nwin) of batch-half i of tile t:
                # 64 descriptors of nwin*FL f32.  2-dim dst with uniform
                # stride only - a 3-dim dst whose shape doesn't match the
                # src AP makes the ucode descriptor generator emit 4B
                # descriptors for the misaligned positions (measured:
                # 15k 4B packets, 2x exec).  64 + 16 sem descs = 80 = 0
                # mod 16, so the engine spray stays uniform in any order.
                return st(nc.gpsimd, yt,
                          [[YROW, HB], [1, nwin * FL]],
                          i * HB * YROW + j0 * FL,
                          [[YROW, HB], [1, nwin * FL]],
                          (2 * t + i) * OB + j0 * FL)

            # ---- loads ----
            # ONE X tile [128, 8*372]: partition 64i+q, column group t
            # holds x[2t+i].flat[248q : 248q+372].  4 load instructions
            # total (2 per HWDGE ring) - a HW queue blocks its engine's
            # instruction FIFO when more than ~4 DMAs are outstanding,
            # which is what delayed the ACT expands to ~29us with
            # per-tile loads (measured).  Tile 0's halves are separate
            # so the first expand's semaphore arrives ~12us.
            XA = xp.tile([128, NT * XROW], mybir.dt.float32)
            XROWS = NT * XROW
            ld(nc.sync, XA, [[XROWS, HB], [1, XROW]], 0,
               [[WPP * C, HB], [1, XROW]], 0)
            ld(nc.sync, XA, [[XROWS, HB], [XROW, NT - 1], [1, XROW]], XROW,
               [[WPP * C, HB], [2 * XB, NT - 1], [1, XROW]], 2 * XB)
            V = vp.tile([16, FL], mybir.dt.float32)
            ld(nc.scalar, V, [[FL, 16], [1, FL]], 0,
               [[XB, B], [1, FL]], W4 * C)
            R = rp.tile([128, RLD], mybir.dt.float32)
            ld(nc.scalar, R, [[RLD, 128], [1, RLD]], 0,
               [[XB, B], [TSTR * C, TSL], [1, RLD]], NBULK * C)
            ld(nc.scalar, XA, [[XROWS, HB], [1, XROW]], HB * XROWS,
               [[WPP * C, HB], [1, XROW]], XB)
            ld(nc.scalar, XA, [[XROWS, HB], [XROW, NT - 1], [1, XROW]],
               HB * XROWS + XROW,
               [[WPP * C, HB], [2 * XB, NT - 1], [1, XROW]], 3 * XB)

            # ---- expands ----
            # DVE: tile-0 head piece, then every tile's second piece.
            # ACT: tile-0 second head piece, tail expand, then tiles
            # 1..7 first pieces.  ~35 us per engine, well under drain.
            Ys = [yp.tile([128, YROW], mybir.dt.float32, name="Yt")
                  for t in range(NT)]
            expandp(nc.vector, XA, 0, Ys[0], 0, H0)
            expandp(nc.scalar, XA, 0, Ys[0], H0, H1)
            T = tp.tile([128, TROW], mybir.dt.float32)
            tsrc = R[:].copy()
            tsrc.ap = mybir.VecI64Pair([[RLD, 128], [C, TWIN], [1, FL]])
            tsrc.offset = 0
            tdst = T[:].copy()
            tdst.ap = mybir.VecI64Pair([[TROW, 128], [FL, TWIN], [1, FL]])
            tdst.offset = 0
            nc.scalar.copy(out=tdst, in_=tsrc)
            for t in range(NT):
                expandp(nc.vector, XA, t, Ys[t], PC, PC)
                if t >= 1:
                    expandp(nc.scalar, XA, t, Ys[t], 0, PC)

            # ---- stores: ALL on GPSIMD/SWDGE, FIFO order by earliest
            # dependency.  Every store: 128 descriptors, disjoint dst.
            st(nc.gpsimd, V, [[FL, 16], [1, FL]], 0,
               [[OB, B], [1, FL]], W4 * FL)
            stp(Ys[0], 0, 0, 0, H0)
            stp(Ys[0], 0, 1, 0, H0)
            stp(Ys[0], 0, 0, H0, H1)
            stp(Ys[0], 0, 1, H0, H1)
            st(nc.gpsimd, T, [[TROW, 128], [1, TROW]], 0,
               [[OB, B], [TSTR * FL, TSL], [1, TROW]], NBULK * FL)
            stp(Ys[0], 0, 0, PC, PC)
            stp(Ys[0], 0, 1, PC, PC)
            # tiles 1-7: one full-row store per batch half.  Both sides
            # merge flat and the lowering re-splits to 32 descriptors of
            # 63488 B, so each engine gets two ADJACENT descriptors =
            # 127 KB of contiguous DRAM writes.  (Piece stores whose
            # descriptor stride exceeds the descriptor size scatter each
            # engine's writes and cost ~35% per-packet rate - measured.)
            for t in range(1, NT):
                for i in range(2):
                    st(nc.gpsimd, Ys[t],
                       [[YROW, HB], [1, YROW]], i * HB * YROW,
                       [[YROW, HB], [1, YROW]], (2 * t + i) * OB)

    nc.finalize()
    return nc


def run_sharded(x: np.ndarray, trace: bool = False):
    """Shard batch across 8 cores, run, gather. Returns (out, raw results)."""
    if "nc" not in _cache:
        _cache["nc"] = build_nc()
    nc = _cache["nc"]

    x = np.ascontiguousarray(x, dtype=np.float32)
    in_maps = [{"x": x[i * B : (i + 1) * B]} for i in range(N_CORES)]
    res = run_bass_kernel_spmd(nc, in_maps, list(range(N_CORES)), trace=trace)
    out = np.concatenate([res.results[i]["out"] for i in range(N_CORES)], axis=0)
    return out, res


def kernel(x: np.ndarray) -> np.ndarray:
    out, _ = run_sharded(x, trace=False)
    return out
